# revision 14
# baseline (speedup 1.0000x reference)
"""TGCN (GCN+GRU temporal) kernel for Trainium2, 8 NeuronCores.

Math refactor of the reference:
  gcn(xt, W, b) = Ahat @ (xt @ W) + b = (Ahat @ xt) @ W + b
with Ahat = D^-1/2 (A + I) D^-1/2 fixed across gates and timesteps.
So: Y = Ahat @ X  (one sparse aggregation over all T*C feature columns),
then per timestep small dense matmuls feed the GRU:
  A_t = Y_t @ Wc_g + bc_g            (Wc_g = W_g @ Wl_g[:64], folded on host)
  Z = sigmoid(A_z + H @ Wl_z[64:]);  R = sigmoid(A_r + H @ Wl_r[64:])
  Ht = tanh(A_h + (R*H) @ Wl_h[64:])
  H = Z*H + (1-Z)*Ht;  acc += p_t * H
  out = sigmoid(acc @ W_o + b_o)

Device mapping (SPMD, 8 cores):
  - Each core owns a contiguous range of dst nodes (N/8).
  - X is cast to bf16, re-laid out t-major with per-step channel pad
    (C=129 -> CP=132). Each core is staged ONLY its own node shard
    (npc, F); the full X is assembled on device via an AllGather
    collective into a Shared-HBM tensor (one staging pass of ~330MB
    total instead of 8x replication).
  - Edges are dst-sorted into 128-dst blocks; per block a fixed number
    of 128-edge "subs". Each sub: indirect-DMA gather of 128 source rows
    (SBUF staging G) + PE matmuls psum_Y += S^T @ G where S is the
    (128 edges x 128 dsts) scaled one-hot (values = edge_norm, self
    loops included). S is built ON DEVICE from compact per-slot
    (dst-local, weight) arrays via one fused tensor_scalar
    (iota == dloc) * w per sub. All PSUM accumulation is fp32.
  - Per block: evacuate psum_Y -> bf16, PE-transpose each timestep's
    channels to channel-major, buffer per node-group (8 blocks = 1024
    nodes), then run the GRU scan on (64 x nodes) tiles.
"""

import os
import numpy as np
import ml_dtypes

BF16 = ml_dtypes.bfloat16
FP8 = ml_dtypes.float8_e4m3

# ---------------- problem constants (hardcoded per the task) ----------------
N_NODES = 50000
N_EDGES = 1600000
IN_CH = 129
OUT_CH = 64
PERIODS = 25
N_CORES = 8
CP = 132                      # per-timestep channel pad (129 real + 3 zero)
F = PERIODS * CP              # 3300 feature columns per node
BLOCK = 128                   # dst nodes per aggregation block
GROUP_BLOCKS = 4              # blocks per GRU node-group


class Cfg:
    """Shape configuration; small instances used for simulator tests."""

    def __init__(self, n_nodes=N_NODES, n_cores=N_CORES, in_ch=IN_CH,
                 periods=PERIODS, out_ch=OUT_CH, subs=None,
                 group_blocks=GROUP_BLOCKS, xdt="fp8"):
        self.xdt = xdt
        assert n_nodes % n_cores == 0
        self.n_nodes = n_nodes
        self.n_cores = n_cores
        self.in_ch = in_ch
        self.periods = periods
        self.out_ch = out_ch
        self.cp = ((in_ch + 3) // 4) * 4  # pad channels to mult of 4
        if self.cp == in_ch:
            self.cp = in_ch + 3  # ensure >= in_ch; keep a small pad
        # channel pieces for transpose/matmul: 128-chunk + remainder
        self.c1 = min(128, self.cp)
        self.c2 = self.cp - self.c1
        self.f = self.periods * self.cp
        self.npc = n_nodes // n_cores           # nodes per core
        self.nblocks = -(-self.npc // BLOCK)    # blocks per core
        self.subs = subs                        # filled from data
        self.group_blocks = group_blocks

    @property
    def key(self):
        return (self.n_nodes, self.n_cores, self.in_ch, self.periods,
                self.out_ch, self.subs, self.group_blocks, self.xdt)


# ---------------------------- host preprocessing ----------------------------

def preprocess(x, edge_index, attention,
               W_z, b_z, Wl_z, bl_z, W_r, b_r, Wl_r, bl_r,
               W_h, b_h, Wl_h, bl_h, W_o, b_o, cfg=None,
               min_subs=0):
    """Build per-core device inputs + replicated weights (pure numpy)."""
    cfg = cfg or Cfg()
    N, C, T = x.shape
    assert N == cfg.n_nodes and C == cfg.in_ch and T == cfg.periods

    src = np.asarray(edge_index[0], dtype=np.int64)
    dst = np.asarray(edge_index[1], dtype=np.int64)

    # GCN symmetric norm with self loops (edge weight 1)
    deg = 1.0 + np.bincount(dst, minlength=N).astype(np.float64)
    dinv = 1.0 / np.sqrt(deg)
    w_edge = (dinv[src] * dinv[dst]).astype(np.float32)

    # append self loops
    allsrc = np.concatenate([src, np.arange(N, dtype=np.int64)])
    alldst = np.concatenate([dst, np.arange(N, dtype=np.int64)])
    allw = np.concatenate([w_edge, (dinv * dinv).astype(np.float32)])

    npc, nb = cfg.npc, cfg.nblocks

    core_of = alldst // npc
    block_of = (alldst % npc) // BLOCK

    # per-(core, block) edge counts -> uniform sub count
    flat = core_of * nb + block_of
    counts = np.bincount(flat, minlength=cfg.n_cores * nb)
    subs = int(-(-counts.max() // BLOCK))
    cfg.subs = max(subs, min_subs, 1)
    S = cfg.subs

    # sort edges by (core, block); order within block irrelevant
    order = np.argsort(flat, kind="stable")
    fs = flat[order]
    ss = allsrc[order]
    ds_ = alldst[order]
    ws = allw[order]

    slots = cfg.n_cores * nb * S * BLOCK
    # slot id for each real edge: (cb * S*BLOCK) + rank within cb
    starts = np.zeros(cfg.n_cores * nb + 1, dtype=np.int64)
    np.cumsum(counts, out=starts[1:])
    rank = np.arange(len(fs)) - starts[fs]
    slot = fs * (S * BLOCK) + rank

    idx_flat = np.zeros(slots, dtype=np.int32)           # gather index (src)
    idx_flat[slot] = ss.astype(np.int32)
    dloc_flat = np.zeros(slots, dtype=np.int32)          # dst within block
    dloc_flat[slot] = ((ds_ % npc) % BLOCK).astype(np.int32)
    w_flat = np.zeros(slots, dtype=np.float32)
    w_flat[slot] = ws

    # per-slot layouts per core: (128 partitions, nb*S) where partition p of
    # sub k holds edge slot k*128+p
    def to_cols(a):
        out = a.reshape(cfg.n_cores, nb * S, BLOCK).transpose(0, 2, 1)
        return np.ascontiguousarray(out)                 # (cores,128,nb*S)

    idx_all = to_cols(idx_flat)
    dloc_all = to_cols(dloc_flat).astype(np.uint8)
    we_all = to_cols(w_flat).astype(BF16)

    # X: bf16/fp8, t-major with per-step pad: X2[n, t*CP + c] = x[n, c, t]
    xnp = BF16 if cfg.xdt == "bf16" else FP8
    x2 = np.zeros((N, cfg.f), dtype=xnp)
    xt = np.transpose(np.asarray(x, dtype=np.float32), (0, 2, 1))  # (N,T,C)
    x2r = x2.reshape(N, cfg.periods, cfg.cp)
    x2r[:, :, :C] = xt.astype(xnp)

    # folded weights
    O = cfg.out_ch
    Wc = np.concatenate([
        np.asarray(W_z, np.float32) @ np.asarray(Wl_z, np.float32)[:O],
        np.asarray(W_r, np.float32) @ np.asarray(Wl_r, np.float32)[:O],
        np.asarray(W_h, np.float32) @ np.asarray(Wl_h, np.float32)[:O],
    ], axis=1)                                            # (C, 3*O)
    Wc_pad = np.zeros((cfg.cp, 3 * O), dtype=np.float32)
    Wc_pad[:C] = Wc
    wc1 = Wc_pad[:cfg.c1].astype(BF16)                    # (c1, 3O)
    wc2 = Wc_pad[cfg.c1:].astype(BF16)                    # (c2, 3O)

    wl2 = np.concatenate([
        np.asarray(Wl_z, np.float32)[O:],
        np.asarray(Wl_r, np.float32)[O:],
        np.asarray(Wl_h, np.float32)[O:],
    ], axis=1).astype(BF16)                               # (O, 3*O)

    bc = np.stack([
        np.asarray(b_z, np.float32) @ np.asarray(Wl_z, np.float32)[:O]
        + np.asarray(bl_z, np.float32),
        np.asarray(b_r, np.float32) @ np.asarray(Wl_r, np.float32)[:O]
        + np.asarray(bl_r, np.float32),
        np.asarray(b_h, np.float32) @ np.asarray(Wl_h, np.float32)[:O]
        + np.asarray(bl_h, np.float32),
    ], axis=1).astype(np.float32)                         # (O, 3)
    bias = np.zeros((O, 4), dtype=np.float32)
    bias[:, :3] = bc
    bias[0, 3] = float(np.asarray(b_o, np.float32).reshape(-1)[0])

    wo = np.asarray(W_o, np.float32).reshape(O, 1).astype(BF16)

    a = np.asarray(attention, np.float32)
    e = np.exp(a - a.max())
    probs = (e / e.sum()).astype(np.float32)              # (T,)

    fh = cfg.f // 2
    per_core = []
    for c in range(cfg.n_cores):
        rows = x2[c * npc:(c + 1) * npc]
        per_core.append({
            "XS0d": np.ascontiguousarray(rows[:, :fh]),
            "XS1d": np.ascontiguousarray(rows[:, fh:]),
            "IDXd": idx_all[c],
            "DLOCd": dloc_all[c],
            "WEd": we_all[c],
            "WC1d": wc1,
            "WC2d": wc2,
            "WL2d": wl2,
            "WOd": wo,
            "BIASd": bias,
        })
    return cfg, per_core, probs


# ------------------------------ kernel builder ------------------------------

def build_nc(cfg, probs):
    import concourse.bass as bass
    import concourse.mybir as mybir
    import concourse.tile as tile
    from concourse import bacc
    from concourse.masks import make_identity

    fp32 = mybir.dt.float32
    bf16 = mybir.dt.bfloat16
    fp8 = mybir.dt.float8e4
    xdt = bf16 if cfg.xdt == "bf16" else fp8
    xbytes = 2 if cfg.xdt == "bf16" else 1
    i32 = mybir.dt.int32
    AF = mybir.ActivationFunctionType
    OP = mybir.AluOpType

    T, O, FF, S, nb = cfg.periods, cfg.out_ch, cfg.f, cfg.subs, cfg.nblocks
    c1, c2, cp = cfg.c1, cfg.c2, cfg.cp

    nc = bacc.Bacc("TRN2", target_bir_lowering=False, debug=False,
                   num_devices=cfg.n_cores)

    FH = FF // 2
    XS0d = nc.dram_tensor("XS0d", (cfg.npc, FH), xdt, kind="ExternalInput")
    XS1d = nc.dram_tensor("XS1d", (cfg.npc, FH), xdt, kind="ExternalInput")
    IDXd = nc.dram_tensor("IDXd", (BLOCK, nb * S), i32, kind="ExternalInput")
    DLOCd = nc.dram_tensor("DLOCd", (BLOCK, nb * S), mybir.dt.uint8,
                           kind="ExternalInput")
    WEd = nc.dram_tensor("WEd", (BLOCK, nb * S), bf16, kind="ExternalInput")
    WC1d = nc.dram_tensor("WC1d", (c1, 3 * O), bf16, kind="ExternalInput")
    WC2d = nc.dram_tensor("WC2d", (c2, 3 * O), bf16, kind="ExternalInput")
    WL2d = nc.dram_tensor("WL2d", (O, 3 * O), bf16, kind="ExternalInput")
    WOd = nc.dram_tensor("WOd", (O, 1), bf16, kind="ExternalInput")
    BIASd = nc.dram_tensor("BIASd", (O, 4), fp32, kind="ExternalInput")
    OUTd = nc.dram_tensor("OUTd", (1, cfg.npc), fp32, kind="ExternalOutput")

    # on-device reassembly of the full X: bounce local shard -> AllGather
    # into Shared-HBM tensors readable by this core's gathers. X is split
    # into two column halves so each tensor fits a 256MB scratchpad page.
    xin0 = nc.dram_tensor("xin0_b", (cfg.npc, FH), xdt, kind="Internal")
    xin1 = nc.dram_tensor("xin1_b", (cfg.npc, FH), xdt, kind="Internal")
    xfull0 = nc.dram_tensor("xfull0", (cfg.n_nodes, FH), xdt,
                            kind="Internal", addr_space="Shared")
    xfull1 = nc.dram_tensor("xfull1", (cfg.n_nodes, FH), xdt,
                            kind="Internal", addr_space="Shared")

    # node groups: lists of block indices
    groups = []
    b = 0
    while b < nb:
        g = list(range(b, min(b + cfg.group_blocks, nb)))
        groups.append(g)
        b += cfg.group_blocks

    MMF = 512  # matmul free-dim chunk

    def fchunks(total, width=MMF):
        out = []
        s0 = 0
        while s0 < total:
            out.append((s0, min(width, total - s0)))
            s0 += width
        return out

    with tile.TileContext(nc) as tc:
        with (
            tc.tile_pool(name="const", bufs=1) as const_p,
            tc.tile_pool(name="spool", bufs=2) as s_p,
            tc.tile_pool(name="gpool", bufs=8) as g_p,
            tc.tile_pool(name="ysb", bufs=2) as ysb_p,
            tc.tile_pool(name="yt", bufs=1) as yt_p,
            tc.tile_pool(name="gru", bufs=1) as gru_p,
            tc.tile_pool(name="outp", bufs=2) as out_p,
            tc.tile_pool(name="psum", bufs=1, space="PSUM") as ps_p,
        ):
            nc.sync.dma_start(xin0[:], XS0d[:])
            nc.sync.dma_start(xin1[:], XS1d[:])
            nc.gpsimd.collective_compute(
                "AllGather", OP.bypass,
                replica_groups=[list(range(cfg.n_cores))],
                ins=[xin0[:].opt()],
                outs=[xfull0[:].opt()],
            )
            nc.gpsimd.collective_compute(
                "AllGather", OP.bypass,
                replica_groups=[list(range(cfg.n_cores))],
                ins=[xin1[:].opt()],
                outs=[xfull1[:].opt()],
            )

            idx_sb = const_p.tile([BLOCK, nb * S], i32)
            nc.sync.dma_start(idx_sb[:], IDXd[:])
            dloc_u8 = const_p.tile([BLOCK, nb * S], mybir.dt.uint8)
            nc.sync.dma_start(dloc_u8[:], DLOCd[:])
            dloc_sb = const_p.tile([BLOCK, nb * S], fp32)
            nc.vector.tensor_copy(out=dloc_sb[:], in_=dloc_u8[:])
            we_bf = const_p.tile([BLOCK, nb * S], bf16)
            nc.sync.dma_start(we_bf[:], WEd[:])
            we_sb = const_p.tile([BLOCK, nb * S], fp32)
            nc.vector.tensor_copy(out=we_sb[:], in_=we_bf[:])
            iota_i = const_p.tile([BLOCK, BLOCK], i32)
            nc.gpsimd.iota(iota_i[:], pattern=[[1, BLOCK]], base=0,
                           channel_multiplier=0)
            iota_bf = const_p.tile([BLOCK, BLOCK], bf16)
            nc.vector.tensor_copy(out=iota_bf[:], in_=iota_i[:])
            wc1_sb = const_p.tile([c1, 3 * O], bf16)
            nc.sync.dma_start(wc1_sb[:], WC1d[:])
            wc2_sb = const_p.tile([c2, 3 * O], bf16)
            nc.sync.dma_start(wc2_sb[:], WC2d[:])
            wl2_sb = const_p.tile([O, 3 * O], bf16)
            nc.sync.dma_start(wl2_sb[:], WL2d[:])
            wo_sb = const_p.tile([O, 1], bf16)
            nc.sync.dma_start(wo_sb[:], WOd[:])
            bias_sb = const_p.tile([O, 4], fp32)
            nc.sync.dma_start(bias_sb[:], BIASd[:])
            ident = const_p.tile([BLOCK, BLOCK], fp32)
            make_identity(nc, ident[:])

            for grp in groups:
                ng = len(grp) * BLOCK          # nodes in group (padded)
                yt1 = yt_p.tile([c1, T, ng], bf16, tag="yt1")
                yt2 = yt_p.tile([max(c2, 1), T, ng], bf16, tag="yt2")

                for bi, blk in enumerate(grp):
                    s_sb = s_p.tile([BLOCK, S * BLOCK], xdt, tag="smat")
                    for s in range(S):
                        col = blk * S + s
                        nc.vector.tensor_scalar(
                            out=s_sb[:, s * BLOCK:(s + 1) * BLOCK],
                            in0=iota_bf[:],
                            scalar1=dloc_sb[:, col:col + 1],
                            scalar2=we_sb[:, col:col + 1],
                            op0=OP.is_equal, op1=OP.mult)
                    ps_y = ps_p.tile([BLOCK, FF], fp32, tag="psy")
                    for s in range(S):
                        col = blk * S + s
                        kw = dict(bounds_check=cfg.n_nodes - 1,
                                  oob_is_err=True)
                        g_sb = g_p.tile([BLOCK, FF], xdt, tag="gath")
                        for hf, xf in ((0, xfull0), (1, xfull1)):
                            nc.gpsimd.indirect_dma_start(
                                out=g_sb[:, hf * FH:(hf + 1) * FH],
                                out_offset=None,
                                in_=xf[:],
                                in_offset=bass.IndirectOffsetOnAxis(
                                    ap=idx_sb[:, col:col + 1], axis=0),
                                **kw,
                            )
                        for f0, fw in fchunks(FF):
                            nc.tensor.matmul(
                                out=ps_y[:, f0:f0 + fw],
                                lhsT=s_sb[:, s * BLOCK:(s + 1) * BLOCK],
                                rhs=g_sb[:, f0:f0 + fw],
                                start=(s == 0),
                                stop=(s == S - 1),
                            )
                    y_sb = ysb_p.tile([BLOCK, FF], fp32, tag="ysb")
                    nc.vector.tensor_copy(out=y_sb[:], in_=ps_y[:])

                    # per-timestep transposes to channel-major
                    for t in range(T):
                        pt = ps_p.tile([128, MMF], fp32, tag="small")
                        nc.tensor.transpose(
                            out=pt[:c1, :BLOCK],
                            in_=y_sb[:, t * cp:t * cp + c1],
                            identity=ident[:],
                        )
                        if c2 > 0:
                            nc.tensor.transpose(
                                out=pt[:c2, BLOCK:2 * BLOCK],
                                in_=y_sb[:, t * cp + c1:t * cp + cp],
                                identity=ident[:],
                            )
                        nc.scalar.activation(
                            out=yt1[:, t, bi * BLOCK:(bi + 1) * BLOCK],
                            in_=pt[:c1, :BLOCK], func=AF.Copy)
                        if c2 > 0:
                            nc.scalar.activation(
                                out=yt2[:, t, bi * BLOCK:(bi + 1) * BLOCK],
                                in_=pt[:c2, BLOCK:2 * BLOCK], func=AF.Copy)

                # ---- GRU scan over this node group ----
                h_f = gru_p.tile([O, ng], fp32, tag="h")
                h_bf = gru_p.tile([O, ng], bf16, tag="hbf")
                acc = gru_p.tile([O, ng], fp32, tag="acc")
                nc.vector.memset(h_f[:], 0)
                nc.vector.memset(h_bf[:], 0)
                nc.vector.memset(acc[:], 0)

                for t in range(T):
                    def gate_psum(gi, rh_tile=None):
                        gs = slice(gi * O, (gi + 1) * O)
                        pa = ps_p.tile([128, MMF], fp32, tag="small")
                        for f0, fw in fchunks(ng):
                            nc.tensor.matmul(
                                out=pa[:O, f0:f0 + fw],
                                lhsT=wc1_sb[:, gs],
                                rhs=yt1[:, t, f0:f0 + fw],
                                start=True, stop=False)
                            if c2 > 0:
                                nc.tensor.matmul(
                                    out=pa[:O, f0:f0 + fw],
                                    lhsT=wc2_sb[:, gs],
                                    rhs=yt2[:, t, f0:f0 + fw],
                                    start=False, stop=False)
                            hsrc = h_bf if rh_tile is None else rh_tile
                            nc.tensor.matmul(
                                out=pa[:O, f0:f0 + fw],
                                lhsT=wl2_sb[:, gs],
                                rhs=hsrc[:, f0:f0 + fw],
                                start=False, stop=True)
                        return pa

                    pz = gate_psum(0)
                    z_t = gru_p.tile([O, ng], fp32, tag="z")
                    nc.scalar.activation(out=z_t[:], in_=pz[:O, :ng],
                                         func=AF.Sigmoid,
                                         bias=bias_sb[:, 0:1])
                    pr = gate_psum(1)
                    r_t = gru_p.tile([O, ng], fp32, tag="r")
                    nc.scalar.activation(out=r_t[:], in_=pr[:O, :ng],
                                         func=AF.Sigmoid,
                                         bias=bias_sb[:, 1:2])
                    rh = gru_p.tile([O, ng], bf16, tag="rh")
                    nc.vector.tensor_tensor(out=rh[:], in0=r_t[:],
                                            in1=h_f[:], op=OP.mult)
                    ph = gate_psum(2, rh_tile=rh)
                    ht = gru_p.tile([O, ng], fp32, tag="ht")
                    nc.scalar.activation(out=ht[:], in_=ph[:O, :ng], func=AF.Tanh,
                                         bias=bias_sb[:, 2:3])
                    # H = Ht + Z*(H - Ht)
                    d_t = gru_p.tile([O, ng], fp32, tag="d")
                    nc.vector.tensor_tensor(out=d_t[:], in0=h_f[:],
                                            in1=ht[:], op=OP.subtract)
                    nc.vector.tensor_tensor(out=d_t[:], in0=z_t[:],
                                            in1=d_t[:], op=OP.mult)
                    nc.vector.tensor_tensor(out=h_f[:], in0=ht[:],
                                            in1=d_t[:], op=OP.add)
                    # acc += p_t * H
                    p_h = gru_p.tile([O, ng], fp32, tag="phh")
                    nc.scalar.activation(out=p_h[:], in_=h_f[:], func=AF.Copy,
                                         scale=float(probs[t]))
                    nc.vector.tensor_tensor(out=acc[:], in0=acc[:],
                                            in1=p_h[:], op=OP.add)
                    if t < T - 1:
                        nc.scalar.activation(out=h_bf[:], in_=h_f[:],
                                             func=AF.Copy)

                # output head
                acc_bf = gru_p.tile([O, ng], bf16, tag="accbf")
                nc.scalar.activation(out=acc_bf[:], in_=acc[:], func=AF.Copy)
                n0 = grp[0] * BLOCK
                for f0, fw in fchunks(ng):
                    po = ps_p.tile([128, MMF], fp32, tag="small")
                    nc.tensor.matmul(out=po[:1, :fw], lhsT=wo_sb[:],
                                     rhs=acc_bf[:, f0:f0 + fw],
                                     start=True, stop=True)
                    o_sb = out_p.tile([1, MMF], fp32, tag="osb")
                    nc.scalar.activation(out=o_sb[:, :fw], in_=po[:1, :fw],
                                         func=AF.Sigmoid,
                                         bias=bias_sb[0:1, 3:4])
                    w0 = n0 + f0
                    w1 = min(n0 + f0 + fw, cfg.npc)
                    if w1 > w0:
                        nc.sync.dma_start(out=OUTd[:, w0:w1],
                                          in_=o_sb[:, :w1 - w0])

    nc.compile()
    return nc


# ------------------------------- entry points -------------------------------

_CACHE = {}


def _get_nc(cfg, probs):
    k = (cfg.key, tuple(np.round(probs, 8).tolist()))
    if k not in _CACHE:
        _CACHE[k] = build_nc(cfg, probs)
    return _CACHE[k]


def run_device(cfg, per_core, probs, trace=False):
    from concourse.bass_utils import run_bass_kernel_spmd
    nc = _get_nc(cfg, probs)
    res = run_bass_kernel_spmd(nc, per_core, core_ids=list(range(cfg.n_cores)),
                               trace=trace)
    outs = [res.results[c]["OUTd"].reshape(-1)[:cfg.npc]
            for c in range(cfg.n_cores)]
    return np.concatenate(outs), res


def kernel(x, edge_index, y, train_idx, attention,
           W_z, b_z, Wl_z, bl_z, W_r, b_r, Wl_r, bl_r,
           W_h, b_h, Wl_h, bl_h, W_o, b_o):
    x = np.asarray(x)
    y = np.asarray(y, dtype=np.float32)
    train_idx = np.asarray(train_idx)
    cfg, per_core, probs = preprocess(
        x, np.asarray(edge_index), np.asarray(attention),
        W_z, b_z, Wl_z, bl_z, W_r, b_r, Wl_r, bl_r,
        W_h, b_h, Wl_h, bl_h, W_o, b_o)
    full, _ = run_device(cfg, per_core, probs,
                         trace=bool(int(os.environ.get("KTRACE", "0"))))
    y_pred = full[train_idx].astype(np.float32)
    return y_pred, y[train_idx]



# revision 15
# speedup vs baseline: 1.9743x; 1.9743x over previous
"""TGCN (GCN+GRU temporal) kernel for Trainium2, 8 NeuronCores.

Math refactor of the reference:
  gcn(xt, W, b) = Ahat @ (xt @ W) + b = (Ahat @ xt) @ W + b
with Ahat = D^-1/2 (A + I) D^-1/2 fixed across gates and timesteps.
So: Y = Ahat @ X  (one sparse aggregation over all T*C feature columns),
then per timestep small dense matmuls feed the GRU:
  A_t = Y_t @ Wc_g + bc_g            (Wc_g = W_g @ Wl_g[:64], folded on host)
  Z = sigmoid(A_z + H @ Wl_z[64:]);  R = sigmoid(A_r + H @ Wl_r[64:])
  Ht = tanh(A_h + (R*H) @ Wl_h[64:])
  H = Z*H + (1-Z)*Ht;  acc += p_t * H
  out = sigmoid(acc @ W_o + b_o)

Device mapping (SPMD, 8 cores):
  - Each core owns a contiguous range of dst nodes (N/8).
  - X is cast to bf16, re-laid out t-major with per-step channel pad
    (C=129 -> CP=132). Each core is staged ONLY its own node shard
    (npc, F); the full X is assembled on device via an AllGather
    collective into a Shared-HBM tensor (one staging pass of ~330MB
    total instead of 8x replication).
  - Edges are dst-sorted into 128-dst blocks; per block a fixed number
    of 128-edge "subs". Each sub: indirect-DMA gather of 128 source rows
    (SBUF staging G) + PE matmuls psum_Y += S^T @ G where S is the
    (128 edges x 128 dsts) scaled one-hot (values = edge_norm, self
    loops included). S is built ON DEVICE from compact per-slot
    (dst-local, weight) arrays via one fused tensor_scalar
    (iota == dloc) * w per sub. All PSUM accumulation is fp32.
  - Per block: evacuate psum_Y -> bf16, PE-transpose each timestep's
    channels to channel-major, buffer per node-group (8 blocks = 1024
    nodes), then run the GRU scan on (64 x nodes) tiles.
"""

import os
import numpy as np
import ml_dtypes

BF16 = ml_dtypes.bfloat16
FP8 = ml_dtypes.float8_e4m3

# ---------------- problem constants (hardcoded per the task) ----------------
N_NODES = 50000
N_EDGES = 1600000
IN_CH = 129
OUT_CH = 64
PERIODS = 25
N_CORES = 8
CP = 132                      # per-timestep channel pad (129 real + 3 zero)
F = PERIODS * CP              # 3300 feature columns per node
BLOCK = 128                   # dst nodes per aggregation block
GROUP_BLOCKS = 4              # blocks per GRU node-group


class Cfg:
    """Shape configuration; small instances used for simulator tests."""

    def __init__(self, n_nodes=N_NODES, n_cores=N_CORES, in_ch=IN_CH,
                 periods=PERIODS, out_ch=OUT_CH, subs=None,
                 group_blocks=GROUP_BLOCKS, xdt="fp8"):
        self.xdt = xdt
        assert n_nodes % n_cores == 0
        self.n_nodes = n_nodes
        self.n_cores = n_cores
        self.in_ch = in_ch
        self.periods = periods
        self.out_ch = out_ch
        self.cp = ((in_ch + 3) // 4) * 4  # pad channels to mult of 4
        if self.cp == in_ch:
            self.cp = in_ch + 3  # ensure >= in_ch; keep a small pad
        # channel pieces for transpose/matmul: 128-chunk + remainder
        self.c1 = min(128, self.cp)
        self.c2 = self.cp - self.c1
        self.f = self.periods * self.cp
        self.npc = n_nodes // n_cores           # nodes per core
        self.nblocks = -(-self.npc // BLOCK)    # blocks per core
        self.subs = subs                        # filled from data
        self.group_blocks = group_blocks

    @property
    def key(self):
        return (self.n_nodes, self.n_cores, self.in_ch, self.periods,
                self.out_ch, self.subs, self.group_blocks, self.xdt)


# ---------------------------- host preprocessing ----------------------------

def preprocess(x, edge_index, attention,
               W_z, b_z, Wl_z, bl_z, W_r, b_r, Wl_r, bl_r,
               W_h, b_h, Wl_h, bl_h, W_o, b_o, cfg=None,
               min_subs=0):
    """Build per-core device inputs + replicated weights (pure numpy)."""
    cfg = cfg or Cfg()
    N, C, T = x.shape
    assert N == cfg.n_nodes and C == cfg.in_ch and T == cfg.periods

    src = np.asarray(edge_index[0], dtype=np.int64)
    dst = np.asarray(edge_index[1], dtype=np.int64)

    # GCN symmetric norm with self loops (edge weight 1)
    deg = 1.0 + np.bincount(dst, minlength=N).astype(np.float64)
    dinv = 1.0 / np.sqrt(deg)
    w_edge = (dinv[src] * dinv[dst]).astype(np.float32)

    # append self loops
    allsrc = np.concatenate([src, np.arange(N, dtype=np.int64)])
    alldst = np.concatenate([dst, np.arange(N, dtype=np.int64)])
    allw = np.concatenate([w_edge, (dinv * dinv).astype(np.float32)])

    npc, nb = cfg.npc, cfg.nblocks

    core_of = alldst // npc
    block_of = (alldst % npc) // BLOCK

    # per-(core, block) edge counts -> uniform sub count
    flat = core_of * nb + block_of
    counts = np.bincount(flat, minlength=cfg.n_cores * nb)
    subs = int(-(-counts.max() // BLOCK))
    cfg.subs = max(subs, min_subs, 1)
    S = cfg.subs

    # sort edges by (core, block); order within block irrelevant
    order = np.argsort(flat, kind="stable")
    fs = flat[order]
    ss = allsrc[order]
    ds_ = alldst[order]
    ws = allw[order]

    slots = cfg.n_cores * nb * S * BLOCK
    # slot id for each real edge: (cb * S*BLOCK) + rank within cb
    starts = np.zeros(cfg.n_cores * nb + 1, dtype=np.int64)
    np.cumsum(counts, out=starts[1:])
    rank = np.arange(len(fs)) - starts[fs]
    slot = fs * (S * BLOCK) + rank

    idx_flat = np.zeros(slots, dtype=np.int32)           # gather index (src)
    idx_flat[slot] = ss.astype(np.int32)
    dloc_flat = np.zeros(slots, dtype=np.int32)          # dst within block
    dloc_flat[slot] = ((ds_ % npc) % BLOCK).astype(np.int32)
    w_flat = np.zeros(slots, dtype=np.float32)
    w_flat[slot] = ws

    # per-slot layouts per core: (128 partitions, nb*S) where partition p of
    # sub k holds edge slot k*128+p
    def to_cols(a):
        out = a.reshape(cfg.n_cores, nb * S, BLOCK).transpose(0, 2, 1)
        return np.ascontiguousarray(out)                 # (cores,128,nb*S)

    idx_all = to_cols(idx_flat)
    dloc_all = to_cols(dloc_flat).astype(np.uint8)
    we_all = to_cols(w_flat).astype(BF16)

    # X: bf16/fp8, t-major with per-step pad: X2[n, t*CP + c] = x[n, c, t]
    xnp = BF16 if cfg.xdt == "bf16" else FP8
    x2 = np.zeros((N, cfg.f), dtype=xnp)
    xt = np.transpose(np.asarray(x, dtype=np.float32), (0, 2, 1))  # (N,T,C)
    x2r = x2.reshape(N, cfg.periods, cfg.cp)
    x2r[:, :, :C] = xt.astype(xnp)

    # folded weights
    O = cfg.out_ch
    Wc = np.concatenate([
        np.asarray(W_z, np.float32) @ np.asarray(Wl_z, np.float32)[:O],
        np.asarray(W_r, np.float32) @ np.asarray(Wl_r, np.float32)[:O],
        np.asarray(W_h, np.float32) @ np.asarray(Wl_h, np.float32)[:O],
    ], axis=1)                                            # (C, 3*O)
    Wc_pad = np.zeros((cfg.cp, 3 * O), dtype=np.float32)
    Wc_pad[:C] = Wc
    wc1 = Wc_pad[:cfg.c1].astype(BF16)                    # (c1, 3O)
    wc2 = Wc_pad[cfg.c1:].astype(BF16)                    # (c2, 3O)

    wl2 = np.concatenate([
        np.asarray(Wl_z, np.float32)[O:],
        np.asarray(Wl_r, np.float32)[O:],
        np.asarray(Wl_h, np.float32)[O:],
    ], axis=1).astype(BF16)                               # (O, 3*O)

    bc = np.stack([
        np.asarray(b_z, np.float32) @ np.asarray(Wl_z, np.float32)[:O]
        + np.asarray(bl_z, np.float32),
        np.asarray(b_r, np.float32) @ np.asarray(Wl_r, np.float32)[:O]
        + np.asarray(bl_r, np.float32),
        np.asarray(b_h, np.float32) @ np.asarray(Wl_h, np.float32)[:O]
        + np.asarray(bl_h, np.float32),
    ], axis=1).astype(np.float32)                         # (O, 3)
    bias = np.zeros((O, 4), dtype=np.float32)
    bias[:, :3] = bc
    bias[0, 3] = float(np.asarray(b_o, np.float32).reshape(-1)[0])

    wo = np.asarray(W_o, np.float32).reshape(O, 1).astype(BF16)

    a = np.asarray(attention, np.float32)
    e = np.exp(a - a.max())
    probs = (e / e.sum()).astype(np.float32)              # (T,)

    fh = cfg.f // 2
    per_core = []
    for c in range(cfg.n_cores):
        rows = x2[c * npc:(c + 1) * npc]
        per_core.append({
            "XS0d": np.ascontiguousarray(rows[:, :fh]),
            "XS1d": np.ascontiguousarray(rows[:, fh:]),
            "IDXd": idx_all[c],
            "DLOCd": dloc_all[c],
            "WEd": we_all[c],
            "WC1d": wc1,
            "WC2d": wc2,
            "WL2d": wl2,
            "WOd": wo,
            "BIASd": bias,
        })
    return cfg, per_core, probs


# ------------------------------ kernel builder ------------------------------

def build_nc(cfg, probs):
    import concourse.bass as bass
    import concourse.mybir as mybir
    import concourse.tile as tile
    from concourse import bacc
    from concourse.masks import make_identity

    fp32 = mybir.dt.float32
    bf16 = mybir.dt.bfloat16
    fp8 = mybir.dt.float8e4
    xdt = bf16 if cfg.xdt == "bf16" else fp8
    xbytes = 2 if cfg.xdt == "bf16" else 1
    i32 = mybir.dt.int32
    AF = mybir.ActivationFunctionType
    OP = mybir.AluOpType

    T, O, FF, S, nb = cfg.periods, cfg.out_ch, cfg.f, cfg.subs, cfg.nblocks
    c1, c2, cp = cfg.c1, cfg.c2, cfg.cp

    nc = bacc.Bacc("TRN2", target_bir_lowering=False, debug=False,
                   num_devices=cfg.n_cores)

    FH = FF // 2
    XS0d = nc.dram_tensor("XS0d", (cfg.npc, FH), xdt, kind="ExternalInput")
    XS1d = nc.dram_tensor("XS1d", (cfg.npc, FH), xdt, kind="ExternalInput")
    IDXd = nc.dram_tensor("IDXd", (BLOCK, nb * S), i32, kind="ExternalInput")
    DLOCd = nc.dram_tensor("DLOCd", (BLOCK, nb * S), mybir.dt.uint8,
                           kind="ExternalInput")
    WEd = nc.dram_tensor("WEd", (BLOCK, nb * S), bf16, kind="ExternalInput")
    WC1d = nc.dram_tensor("WC1d", (c1, 3 * O), bf16, kind="ExternalInput")
    WC2d = nc.dram_tensor("WC2d", (c2, 3 * O), bf16, kind="ExternalInput")
    WL2d = nc.dram_tensor("WL2d", (O, 3 * O), bf16, kind="ExternalInput")
    WOd = nc.dram_tensor("WOd", (O, 1), bf16, kind="ExternalInput")
    BIASd = nc.dram_tensor("BIASd", (O, 4), fp32, kind="ExternalInput")
    OUTd = nc.dram_tensor("OUTd", (1, cfg.npc), fp32, kind="ExternalOutput")

    # on-device reassembly of the full X: bounce local shard -> AllGather
    # into Shared-HBM tensors readable by this core's gathers. X is split
    # into two column halves so each tensor fits a 256MB scratchpad page.
    xin0 = nc.dram_tensor("xin0_b", (cfg.npc, FH), xdt, kind="Internal")
    xin1 = nc.dram_tensor("xin1_b", (cfg.npc, FH), xdt, kind="Internal")
    xfull0 = nc.dram_tensor("xfull0", (cfg.n_nodes, FH), xdt,
                            kind="Internal", addr_space="Shared")
    xfull1 = nc.dram_tensor("xfull1", (cfg.n_nodes, FH), xdt,
                            kind="Internal", addr_space="Shared")

    # node groups: lists of block indices
    groups = []
    b = 0
    while b < nb:
        g = list(range(b, min(b + cfg.group_blocks, nb)))
        groups.append(g)
        b += cfg.group_blocks

    MMF = 512  # matmul free-dim chunk

    def fchunks(total, width=MMF):
        out = []
        s0 = 0
        while s0 < total:
            out.append((s0, min(width, total - s0)))
            s0 += width
        return out

    with tile.TileContext(nc) as tc:
        with (
            tc.tile_pool(name="const", bufs=1) as const_p,
            tc.tile_pool(name="spool", bufs=2) as s_p,
            tc.tile_pool(name="gpool", bufs=8) as g_p,
            tc.tile_pool(name="ysb", bufs=2) as ysb_p,
            tc.tile_pool(name="yt", bufs=1) as yt_p,
            tc.tile_pool(name="gru", bufs=1) as gru_p,
            tc.tile_pool(name="outp", bufs=2) as out_p,
            tc.tile_pool(name="psum", bufs=1, space="PSUM") as ps_p,
        ):
            nc.sync.dma_start(xin0[:], XS0d[:])
            nc.sync.dma_start(xin1[:], XS1d[:])
            nc.gpsimd.collective_compute(
                "AllGather", OP.bypass,
                replica_groups=[list(range(cfg.n_cores))],
                ins=[xin0[:].opt()],
                outs=[xfull0[:].opt()],
            )
            nc.gpsimd.collective_compute(
                "AllGather", OP.bypass,
                replica_groups=[list(range(cfg.n_cores))],
                ins=[xin1[:].opt()],
                outs=[xfull1[:].opt()],
            )

            idx_sb = const_p.tile([BLOCK, nb * S], i32)
            nc.sync.dma_start(idx_sb[:], IDXd[:])
            dloc_u8 = const_p.tile([BLOCK, nb * S], mybir.dt.uint8)
            nc.sync.dma_start(dloc_u8[:], DLOCd[:])
            dloc_sb = const_p.tile([BLOCK, nb * S], fp32)
            nc.vector.tensor_copy(out=dloc_sb[:], in_=dloc_u8[:])
            we_bf = const_p.tile([BLOCK, nb * S], bf16)
            nc.sync.dma_start(we_bf[:], WEd[:])
            we_sb = const_p.tile([BLOCK, nb * S], fp32)
            nc.vector.tensor_copy(out=we_sb[:], in_=we_bf[:])
            iota_i = const_p.tile([BLOCK, BLOCK], i32)
            nc.gpsimd.iota(iota_i[:], pattern=[[1, BLOCK]], base=0,
                           channel_multiplier=0)
            iota_bf = const_p.tile([BLOCK, BLOCK], bf16)
            nc.vector.tensor_copy(out=iota_bf[:], in_=iota_i[:])
            wc1_sb = const_p.tile([c1, 3 * O], bf16)
            nc.sync.dma_start(wc1_sb[:], WC1d[:])
            wc2_sb = const_p.tile([c2, 3 * O], bf16)
            nc.sync.dma_start(wc2_sb[:], WC2d[:])
            wl2_sb = const_p.tile([O, 3 * O], bf16)
            nc.sync.dma_start(wl2_sb[:], WL2d[:])
            wo_sb = const_p.tile([O, 1], bf16)
            nc.sync.dma_start(wo_sb[:], WOd[:])
            bias_sb = const_p.tile([O, 4], fp32)
            nc.sync.dma_start(bias_sb[:], BIASd[:])
            ident = const_p.tile([BLOCK, BLOCK], fp32)
            make_identity(nc, ident[:])

            for grp in groups:
                ng = len(grp) * BLOCK          # nodes in group (padded)
                yt1 = yt_p.tile([c1, T, ng], bf16, tag="yt1")
                yt2 = yt_p.tile([max(c2, 1), T, ng], bf16, tag="yt2")

                for bi, blk in enumerate(grp):
                    s_sb = s_p.tile([BLOCK, S * BLOCK], xdt, tag="smat")
                    for s in range(S):
                        col = blk * S + s
                        nc.vector.tensor_scalar(
                            out=s_sb[:, s * BLOCK:(s + 1) * BLOCK],
                            in0=iota_bf[:],
                            scalar1=dloc_sb[:, col:col + 1],
                            scalar2=we_sb[:, col:col + 1],
                            op0=OP.is_equal, op1=OP.mult)
                    ps_y = ps_p.tile([BLOCK, FF], fp32, tag="psy")
                    for s in range(S):
                        col = blk * S + s
                        kw = dict(bounds_check=cfg.n_nodes - 1,
                                  oob_is_err=True)
                        g_sb = g_p.tile([BLOCK, FF], xdt, tag="gath")
                        for hf, xf in ((0, xfull0), (1, xfull1)):
                            nc.gpsimd.indirect_dma_start(
                                out=g_sb[:, hf * FH:(hf + 1) * FH],
                                out_offset=None,
                                in_=xf[:],
                                in_offset=bass.IndirectOffsetOnAxis(
                                    ap=idx_sb[:, col:col + 1], axis=0),
                                **kw,
                            )
                        for f0, fw in fchunks(FF):
                            nc.tensor.matmul(
                                out=ps_y[:, f0:f0 + fw],
                                lhsT=s_sb[:, s * BLOCK:(s + 1) * BLOCK],
                                rhs=g_sb[:, f0:f0 + fw],
                                start=(s == 0),
                                stop=(s == S - 1),
                            )
                    y_sb = ysb_p.tile([BLOCK, FF], fp32, tag="ysb")
                    nc.vector.tensor_copy(out=y_sb[:], in_=ps_y[:])

                    # per-timestep transposes to channel-major
                    for t in range(T):
                        pt = ps_p.tile([128, MMF], fp32, tag="small")
                        nc.tensor.transpose(
                            out=pt[:c1, :BLOCK],
                            in_=y_sb[:, t * cp:t * cp + c1],
                            identity=ident[:],
                        )
                        if c2 > 0:
                            nc.tensor.transpose(
                                out=pt[:c2, BLOCK:2 * BLOCK],
                                in_=y_sb[:, t * cp + c1:t * cp + cp],
                                identity=ident[:],
                            )
                        nc.scalar.activation(
                            out=yt1[:, t, bi * BLOCK:(bi + 1) * BLOCK],
                            in_=pt[:c1, :BLOCK], func=AF.Copy)
                        if c2 > 0:
                            nc.scalar.activation(
                                out=yt2[:, t, bi * BLOCK:(bi + 1) * BLOCK],
                                in_=pt[:c2, BLOCK:2 * BLOCK], func=AF.Copy)

                # ---- GRU scan over this node group ----
                h_f = gru_p.tile([O, ng], fp32, tag="h")
                h_bf = gru_p.tile([O, ng], bf16, tag="hbf")
                acc = gru_p.tile([O, ng], fp32, tag="acc")
                nc.vector.memset(h_f[:], 0)
                nc.vector.memset(h_bf[:], 0)
                nc.vector.memset(acc[:], 0)

                for t in range(T):
                    def gate_psum(gi, rh_tile=None):
                        gs = slice(gi * O, (gi + 1) * O)
                        pa = ps_p.tile([128, MMF], fp32, tag="small")
                        for f0, fw in fchunks(ng):
                            nc.tensor.matmul(
                                out=pa[:O, f0:f0 + fw],
                                lhsT=wc1_sb[:, gs],
                                rhs=yt1[:, t, f0:f0 + fw],
                                start=True, stop=False)
                            if c2 > 0:
                                nc.tensor.matmul(
                                    out=pa[:O, f0:f0 + fw],
                                    lhsT=wc2_sb[:, gs],
                                    rhs=yt2[:, t, f0:f0 + fw],
                                    start=False, stop=False)
                            hsrc = h_bf if rh_tile is None else rh_tile
                            nc.tensor.matmul(
                                out=pa[:O, f0:f0 + fw],
                                lhsT=wl2_sb[:, gs],
                                rhs=hsrc[:, f0:f0 + fw],
                                start=False, stop=True)
                        return pa

                    pz = gate_psum(0)
                    z_t = gru_p.tile([O, ng], fp32, tag="z")
                    nc.scalar.activation(out=z_t[:], in_=pz[:O, :ng],
                                         func=AF.Sigmoid,
                                         bias=bias_sb[:, 0:1])
                    pr = gate_psum(1)
                    r_t = gru_p.tile([O, ng], fp32, tag="r")
                    nc.scalar.activation(out=r_t[:], in_=pr[:O, :ng],
                                         func=AF.Sigmoid,
                                         bias=bias_sb[:, 1:2])
                    rh = gru_p.tile([O, ng], bf16, tag="rh")
                    nc.vector.tensor_tensor(out=rh[:], in0=r_t[:],
                                            in1=h_f[:], op=OP.mult)
                    ph = gate_psum(2, rh_tile=rh)
                    ht = gru_p.tile([O, ng], fp32, tag="ht")
                    nc.scalar.activation(out=ht[:], in_=ph[:O, :ng], func=AF.Tanh,
                                         bias=bias_sb[:, 2:3])
                    # H = Ht + Z*(H - Ht)
                    d_t = gru_p.tile([O, ng], fp32, tag="d")
                    nc.vector.tensor_tensor(out=d_t[:], in0=h_f[:],
                                            in1=ht[:], op=OP.subtract)
                    nc.vector.tensor_tensor(out=d_t[:], in0=z_t[:],
                                            in1=d_t[:], op=OP.mult)
                    nc.vector.tensor_tensor(out=h_f[:], in0=ht[:],
                                            in1=d_t[:], op=OP.add)
                    # acc += p_t * H
                    p_h = gru_p.tile([O, ng], fp32, tag="phh")
                    nc.scalar.activation(out=p_h[:], in_=h_f[:], func=AF.Copy,
                                         scale=float(probs[t]))
                    nc.vector.tensor_tensor(out=acc[:], in0=acc[:],
                                            in1=p_h[:], op=OP.add)
                    if t < T - 1:
                        nc.scalar.activation(out=h_bf[:], in_=h_f[:],
                                             func=AF.Copy)

                # output head
                acc_bf = gru_p.tile([O, ng], bf16, tag="accbf")
                nc.scalar.activation(out=acc_bf[:], in_=acc[:], func=AF.Copy)
                n0 = grp[0] * BLOCK
                for f0, fw in fchunks(ng):
                    po = ps_p.tile([128, MMF], fp32, tag="small")
                    nc.tensor.matmul(out=po[:1, :fw], lhsT=wo_sb[:],
                                     rhs=acc_bf[:, f0:f0 + fw],
                                     start=True, stop=True)
                    o_sb = out_p.tile([1, MMF], fp32, tag="osb")
                    nc.scalar.activation(out=o_sb[:, :fw], in_=po[:1, :fw],
                                         func=AF.Sigmoid,
                                         bias=bias_sb[0:1, 3:4])
                    w0 = n0 + f0
                    w1 = min(n0 + f0 + fw, cfg.npc)
                    if w1 > w0:
                        nc.sync.dma_start(out=OUTd[:, w0:w1],
                                          in_=o_sb[:, :w1 - w0])

    nc.compile()
    return nc


# ------------------------------- entry points -------------------------------

_CACHE = {}


def _get_nc(cfg, probs):
    k = (cfg.key, tuple(np.round(probs, 8).tolist()))
    if k not in _CACHE:
        _CACHE[k] = build_nc(cfg, probs)
    return _CACHE[k]


_RUNNER_CACHE = {}


def _get_runner(nc, n_cores):
    """Build (once) a reusable jitted SPMD executor for `nc`.

    Mirrors concourse.bass2jax.run_bass_via_pjrt, but caches the jitted
    callable so warm calls skip XLA/BIR re-compilation (which otherwise
    costs seconds per call).
    """
    key = id(nc)
    if key in _RUNNER_CACHE:
        return _RUNNER_CACHE[key]

    import jax
    from jax.sharding import Mesh, PartitionSpec
    from jax.experimental.shard_map import shard_map
    from concourse import bass2jax, mybir

    bass2jax.install_neuronx_cc_hook()
    assert nc.dbg_addr is None or not nc.dbg_callbacks

    partition_name = (nc.partition_id_tensor.name
                      if nc.partition_id_tensor else None)
    in_names, out_names, out_avals = [], [], []
    for alloc in nc.m.functions[0].allocations:
        if not isinstance(alloc, mybir.MemoryLocationSet):
            continue
        name = alloc.memorylocations[0].name
        if alloc.kind == "ExternalInput":
            if name != partition_name and name != (
                    nc.dbg_addr.name if nc.dbg_addr is not None else None):
                in_names.append(name)
        elif alloc.kind == "ExternalOutput":
            out_names.append(name)
            out_avals.append(jax.core.ShapedArray(
                tuple(alloc.tensor_shape), mybir.dt.np(alloc.dtype)))
    n_params = len(in_names)
    body_in_names = list(in_names) + list(out_names)
    if nc.dbg_addr is not None:
        body_in_names.append(nc.dbg_addr.name)
    if partition_name is not None:
        body_in_names.append(partition_name)

    donate = tuple(range(n_params, n_params + len(out_names)))

    def _body(*args):
        operands = list(args)
        if nc.dbg_addr is not None:
            operands.append(jax.numpy.zeros((1, 2), np.uint32))
        if partition_name is not None:
            operands.append(bass2jax.partition_id_tensor())
        outs = bass2jax._bass_exec_p.bind(
            *operands,
            out_avals=tuple(out_avals),
            in_names=tuple(body_in_names),
            out_names=tuple(out_names),
            lowering_input_output_aliases=(),
            sim_require_finite=True,
            sim_require_nnan=True,
            nc=nc,
        )
        return tuple(outs)

    devices = jax.devices()[:n_cores]
    mesh = Mesh(np.asarray(devices), ("core",))
    in_specs = (PartitionSpec("core"),) * (n_params + len(out_names))
    out_specs = (PartitionSpec("core"),) * len(out_names)
    sharded = jax.jit(
        shard_map(_body, mesh=mesh, in_specs=in_specs, out_specs=out_specs,
                  check_rep=False),
        donate_argnums=donate, keep_unused=True)

    def run(in_maps):
        concat_in = [
            np.concatenate([np.asarray(m[name]) for m in in_maps], axis=0)
            for name in in_names
        ]
        concat_zeros = [
            np.zeros((n_cores * a.shape[0], *a.shape[1:]), a.dtype)
            for a in out_avals
        ]
        out_arrs = sharded(*concat_in, *concat_zeros)
        return [
            {name: np.asarray(out_arrs[i]).reshape(
                n_cores, *out_avals[i].shape)[c]
             for i, name in enumerate(out_names)}
            for c in range(n_cores)
        ]

    _RUNNER_CACHE[key] = run
    return run


def run_device(cfg, per_core, probs, trace=False):
    nc = _get_nc(cfg, probs)
    if trace:
        from concourse.bass_utils import run_bass_kernel_spmd
        res = run_bass_kernel_spmd(nc, per_core,
                                   core_ids=list(range(cfg.n_cores)),
                                   trace=True)
        results = res.results
    else:
        results = _get_runner(nc, cfg.n_cores)(per_core)
        res = None
    outs = [results[c]["OUTd"].reshape(-1)[:cfg.npc]
            for c in range(cfg.n_cores)]
    return np.concatenate(outs), res


def kernel(x, edge_index, y, train_idx, attention,
           W_z, b_z, Wl_z, bl_z, W_r, b_r, Wl_r, bl_r,
           W_h, b_h, Wl_h, bl_h, W_o, b_o):
    x = np.asarray(x)
    y = np.asarray(y, dtype=np.float32)
    train_idx = np.asarray(train_idx)
    cfg, per_core, probs = preprocess(
        x, np.asarray(edge_index), np.asarray(attention),
        W_z, b_z, Wl_z, bl_z, W_r, b_r, Wl_r, bl_r,
        W_h, b_h, Wl_h, bl_h, W_o, b_o)
    full, _ = run_device(cfg, per_core, probs,
                         trace=bool(int(os.environ.get("KTRACE", "0"))))
    y_pred = full[train_idx].astype(np.float32)
    return y_pred, y[train_idx]



# revision 19
# speedup vs baseline: 19.7658x; 10.0115x over previous
"""TGCN (GCN+GRU temporal) kernel for Trainium2, 8 NeuronCores.

Math refactor of the reference:
  gcn(xt, W, b) = Ahat @ (xt @ W) + b = (Ahat @ xt) @ W + b
with Ahat = D^-1/2 (A + I) D^-1/2 fixed across gates and timesteps.
So: Y = Ahat @ X  (one sparse aggregation over all T*C feature columns),
then per timestep small dense matmuls feed the GRU:
  A_t = Y_t @ Wc_g + bc_g            (Wc_g = W_g @ Wl_g[:64], folded on host)
  Z = sigmoid(A_z + H @ Wl_z[64:]);  R = sigmoid(A_r + H @ Wl_r[64:])
  Ht = tanh(A_h + (R*H) @ Wl_h[64:])
  H = Z*H + (1-Z)*Ht;  acc += p_t * H
  out = sigmoid(acc @ W_o + b_o)

Device mapping (SPMD, 8 cores):
  - Each core owns a contiguous range of dst nodes (N/8).
  - X is cast to bf16, re-laid out t-major with per-step channel pad
    (C=129 -> CP=132). Each core is staged ONLY its own node shard
    (npc, F); the full X is assembled on device via an AllGather
    collective into a Shared-HBM tensor (one staging pass of ~330MB
    total instead of 8x replication).
  - Edges are dst-sorted into 128-dst blocks; per block a fixed number
    of 128-edge "subs". Each sub: indirect-DMA gather of 128 source rows
    (SBUF staging G) + PE matmuls psum_Y += S^T @ G where S is the
    (128 edges x 128 dsts) scaled one-hot (values = edge_norm, self
    loops included). S is built ON DEVICE from compact per-slot
    (dst-local, weight) arrays via one fused tensor_scalar
    (iota == dloc) * w per sub. All PSUM accumulation is fp32.
  - Per block: evacuate psum_Y -> bf16, PE-transpose each timestep's
    channels to channel-major, buffer per node-group (8 blocks = 1024
    nodes), then run the GRU scan on (64 x nodes) tiles.
"""

import os
import numpy as np
import ml_dtypes

BF16 = ml_dtypes.bfloat16
FP8 = ml_dtypes.float8_e4m3
INT4_S = 0.75                 # int4 quant step (covers +-6 sigma)

# ---------------- problem constants (hardcoded per the task) ----------------
N_NODES = 50000
N_EDGES = 1600000
IN_CH = 129
OUT_CH = 64
PERIODS = 25
N_CORES = 8
CP = 132                      # per-timestep channel pad (129 real + 3 zero)
F = PERIODS * CP              # 3300 feature columns per node
BLOCK = 128                   # dst nodes per aggregation block
GROUP_BLOCKS = 4              # blocks per GRU node-group


class Cfg:
    """Shape configuration; small instances used for simulator tests."""

    def __init__(self, n_nodes=N_NODES, n_cores=N_CORES, in_ch=IN_CH,
                 periods=PERIODS, out_ch=OUT_CH, subs=None,
                 group_blocks=GROUP_BLOCKS, xdt="int4"):
        self.xdt = xdt
        assert n_nodes % n_cores == 0
        self.n_nodes = n_nodes
        self.n_cores = n_cores
        self.in_ch = in_ch
        self.periods = periods
        self.out_ch = out_ch
        self.cp = ((in_ch + 3) // 4) * 4  # pad channels to mult of 4
        if self.cp == in_ch:
            self.cp = in_ch + 3  # ensure >= in_ch; keep a small pad
        # channel pieces for transpose/matmul: 128-chunk + remainder
        self.c1 = min(128, self.cp)
        self.c2 = self.cp - self.c1
        self.f = self.periods * self.cp
        self.npc = n_nodes // n_cores           # nodes per core
        self.nblocks = -(-self.npc // BLOCK)    # blocks per core
        self.subs = subs                        # filled from data
        self.group_blocks = group_blocks

    @property
    def key(self):
        return (self.n_nodes, self.n_cores, self.in_ch, self.periods,
                self.out_ch, self.subs, self.group_blocks, self.xdt)


# ---------------------------- host preprocessing ----------------------------

def preprocess(x, edge_index, attention,
               W_z, b_z, Wl_z, bl_z, W_r, b_r, Wl_r, bl_r,
               W_h, b_h, Wl_h, bl_h, W_o, b_o, cfg=None,
               min_subs=0):
    """Build per-core device inputs + replicated weights (pure numpy)."""
    cfg = cfg or Cfg()
    N, C, T = x.shape
    assert N == cfg.n_nodes and C == cfg.in_ch and T == cfg.periods

    src = np.asarray(edge_index[0], dtype=np.int64)
    dst = np.asarray(edge_index[1], dtype=np.int64)

    # GCN symmetric norm with self loops (edge weight 1)
    deg = 1.0 + np.bincount(dst, minlength=N).astype(np.float64)
    dinv = 1.0 / np.sqrt(deg)
    w_edge = (dinv[src] * dinv[dst]).astype(np.float32)

    # append self loops
    allsrc = np.concatenate([src, np.arange(N, dtype=np.int64)])
    alldst = np.concatenate([dst, np.arange(N, dtype=np.int64)])
    allw = np.concatenate([w_edge, (dinv * dinv).astype(np.float32)])

    npc, nb = cfg.npc, cfg.nblocks

    core_of = alldst // npc
    block_of = (alldst % npc) // BLOCK

    # per-(core, block) edge counts -> uniform sub count
    flat = core_of * nb + block_of
    counts = np.bincount(flat, minlength=cfg.n_cores * nb)
    subs = int(-(-counts.max() // BLOCK))
    cfg.subs = max(subs, min_subs, 1)
    S = cfg.subs

    # sort edges by (core, block); order within block irrelevant
    order = np.argsort(flat, kind="stable")
    fs = flat[order]
    ss = allsrc[order]
    ds_ = alldst[order]
    ws = allw[order]

    slots = cfg.n_cores * nb * S * BLOCK
    # slot id for each real edge: (cb * S*BLOCK) + rank within cb
    starts = np.zeros(cfg.n_cores * nb + 1, dtype=np.int64)
    np.cumsum(counts, out=starts[1:])
    rank = np.arange(len(fs)) - starts[fs]
    slot = fs * (S * BLOCK) + rank

    idx_flat = np.zeros(slots, dtype=np.int32)           # gather index (src)
    idx_flat[slot] = ss.astype(np.int32)
    dloc_flat = np.zeros(slots, dtype=np.int32)          # dst within block
    dloc_flat[slot] = ((ds_ % npc) % BLOCK).astype(np.int32)
    w_flat = np.zeros(slots, dtype=np.float32)
    w_flat[slot] = ws

    # per-slot layouts per core: (128 partitions, nb*S) where partition p of
    # sub k holds edge slot k*128+p
    def to_cols(a):
        out = a.reshape(cfg.n_cores, nb * S, BLOCK).transpose(0, 2, 1)
        return np.ascontiguousarray(out)                 # (cores,128,nb*S)

    idx_all = to_cols(idx_flat).astype(np.uint16)
    dloc_all = to_cols(dloc_flat).astype(np.uint8)
    we_all = to_cols(w_flat).astype(BF16)

    # X: t-major with per-step pad: X2[n, t*CP + c] = x[n, c, t]
    xt = np.transpose(np.asarray(x, dtype=np.float32), (0, 2, 1))  # (N,T,C)
    if cfg.xdt == "int4":
        # 4-bit quantization, scale INT4_S covering +-6 sigma. Byte j of
        # timestep block t packs channels (j, j+cp/2): lo=j, hi=j+cp/2.
        cph = cfg.cp // 2
        q = np.zeros((N, cfg.periods, cfg.cp), dtype=np.int16)
        q[:, :, :C] = np.clip(np.rint(xt / INT4_S), -8, 7).astype(np.int16)
        n4 = (q + 8).astype(np.uint8)
        x2 = (n4[:, :, :cph] | (n4[:, :, cph:] << 4))  # (N, T, cph) u8
    else:
        xnp = BF16 if cfg.xdt == "bf16" else FP8
        x2 = np.zeros((N, cfg.f), dtype=xnp)
        x2r = x2.reshape(N, cfg.periods, cfg.cp)
        x2r[:, :, :C] = xt.astype(xnp)

    # folded weights
    O = cfg.out_ch
    Wc = np.concatenate([
        np.asarray(W_z, np.float32) @ np.asarray(Wl_z, np.float32)[:O],
        np.asarray(W_r, np.float32) @ np.asarray(Wl_r, np.float32)[:O],
        np.asarray(W_h, np.float32) @ np.asarray(Wl_h, np.float32)[:O],
    ], axis=1)                                            # (C, 3*O)
    Wc_pad = np.zeros((cfg.cp, 3 * O), dtype=np.float32)
    Wc_pad[:C] = Wc
    wc1 = Wc_pad[:cfg.c1].astype(BF16)                    # (c1, 3O)
    wc2 = Wc_pad[cfg.c1:].astype(BF16)                    # (c2, 3O)

    wl2 = np.concatenate([
        np.asarray(Wl_z, np.float32)[O:],
        np.asarray(Wl_r, np.float32)[O:],
        np.asarray(Wl_h, np.float32)[O:],
    ], axis=1).astype(BF16)                               # (O, 3*O)

    bc = np.stack([
        np.asarray(b_z, np.float32) @ np.asarray(Wl_z, np.float32)[:O]
        + np.asarray(bl_z, np.float32),
        np.asarray(b_r, np.float32) @ np.asarray(Wl_r, np.float32)[:O]
        + np.asarray(bl_r, np.float32),
        np.asarray(b_h, np.float32) @ np.asarray(Wl_h, np.float32)[:O]
        + np.asarray(bl_h, np.float32),
    ], axis=1).astype(np.float32)                         # (O, 3)
    bias = np.zeros((O, 4), dtype=np.float32)
    bias[:, :3] = bc
    bias[0, 3] = float(np.asarray(b_o, np.float32).reshape(-1)[0])

    wo = np.asarray(W_o, np.float32).reshape(O, 1).astype(BF16)

    a = np.asarray(attention, np.float32)
    e = np.exp(a - a.max())
    probs = (e / e.sum()).astype(np.float32)              # (T,)

    per_core = []
    for c in range(cfg.n_cores):
        rows = x2[c * npc:(c + 1) * npc]
        if cfg.xdt == "int4":
            xs = {"XSPd": np.ascontiguousarray(rows)}
        else:
            fh = cfg.f // 2
            xs = {"XS0d": np.ascontiguousarray(rows[:, :fh]),
                  "XS1d": np.ascontiguousarray(rows[:, fh:])}
        per_core.append({
            **xs,
            "IDXd": idx_all[c],
            "DLOCd": dloc_all[c],
            "WEd": we_all[c],
            "WC1d": wc1,
            "WC2d": wc2,
            "WL2d": wl2,
            "WOd": wo,
            "BIASd": bias,
        })
    return cfg, per_core, probs


# ------------------------------ kernel builder ------------------------------

def build_nc(cfg, probs):
    import concourse.bass as bass
    import concourse.mybir as mybir
    import concourse.tile as tile
    from concourse import bacc
    from concourse.masks import make_identity

    fp32 = mybir.dt.float32
    bf16 = mybir.dt.bfloat16
    fp8 = mybir.dt.float8e4
    xdt = bf16 if cfg.xdt == "bf16" else fp8
    xbytes = 2 if cfg.xdt == "bf16" else 1
    i32 = mybir.dt.int32
    AF = mybir.ActivationFunctionType
    OP = mybir.AluOpType

    T, O, FF, S, nb = cfg.periods, cfg.out_ch, cfg.f, cfg.subs, cfg.nblocks
    c1, c2, cp = cfg.c1, cfg.c2, cfg.cp

    nc = bacc.Bacc("TRN2", target_bir_lowering=False, debug=False,
                   num_devices=cfg.n_cores)

    u8 = mybir.dt.uint8
    u16 = mybir.dt.uint16
    if cfg.xdt == "int4":
        cph = cp // 2
        XSPd = nc.dram_tensor("XSPd", (cfg.npc, T, cph), u8,
                              kind="ExternalInput")
    else:
        FH = FF // 2
        XS0d = nc.dram_tensor("XS0d", (cfg.npc, FH), xdt,
                              kind="ExternalInput")
        XS1d = nc.dram_tensor("XS1d", (cfg.npc, FH), xdt,
                              kind="ExternalInput")
    IDXd = nc.dram_tensor("IDXd", (BLOCK, nb * S), u16, kind="ExternalInput")
    DLOCd = nc.dram_tensor("DLOCd", (BLOCK, nb * S), mybir.dt.uint8,
                           kind="ExternalInput")
    WEd = nc.dram_tensor("WEd", (BLOCK, nb * S), bf16, kind="ExternalInput")
    WC1d = nc.dram_tensor("WC1d", (c1, 3 * O), bf16, kind="ExternalInput")
    WC2d = nc.dram_tensor("WC2d", (c2, 3 * O), bf16, kind="ExternalInput")
    WL2d = nc.dram_tensor("WL2d", (O, 3 * O), bf16, kind="ExternalInput")
    WOd = nc.dram_tensor("WOd", (O, 1), bf16, kind="ExternalInput")
    BIASd = nc.dram_tensor("BIASd", (O, 4), fp32, kind="ExternalInput")
    OUTd = nc.dram_tensor("OUTd", (1, cfg.npc), fp32, kind="ExternalOutput")

    # on-device reassembly of the full X: bounce local shard -> AllGather
    # into Shared-HBM tensors readable by this core's gathers. For 2-byte
    # X the full tensor exceeds the 256MB scratchpad page, so it is split
    # into two column halves; 1-byte X uses a single tensor. int4 unpacks
    # the staged nibbles to fp8 before the AllGather.
    if cfg.xdt == "int4":
        xin = nc.dram_tensor("xin_b", (cfg.npc, T, cp), fp8, kind="Internal")
        xfull = nc.dram_tensor("xfull", (cfg.n_nodes, FF), fp8,
                               kind="Internal", addr_space="Shared")
        halves = [(xfull, 0, FF)]
    else:
        xin0 = nc.dram_tensor("xin0_b", (cfg.npc, FH), xdt, kind="Internal")
        xin1 = nc.dram_tensor("xin1_b", (cfg.npc, FH), xdt, kind="Internal")
        xfull0 = nc.dram_tensor("xfull0", (cfg.n_nodes, FH), xdt,
                                kind="Internal", addr_space="Shared")
        xfull1 = nc.dram_tensor("xfull1", (cfg.n_nodes, FH), xdt,
                                kind="Internal", addr_space="Shared")
        halves = [(xfull0, 0, FH), (xfull1, FH, FH)]

    # node groups: lists of block indices
    groups = []
    b = 0
    while b < nb:
        g = list(range(b, min(b + cfg.group_blocks, nb)))
        groups.append(g)
        b += cfg.group_blocks

    MMF = 512  # matmul free-dim chunk

    def fchunks(total, width=MMF):
        out = []
        s0 = 0
        while s0 < total:
            out.append((s0, min(width, total - s0)))
            s0 += width
        return out

    with tile.TileContext(nc) as tc:
        with (
            tc.tile_pool(name="const", bufs=1) as const_p,
            tc.tile_pool(name="spool", bufs=2) as s_p,
            tc.tile_pool(name="gpool", bufs=8) as g_p,
            tc.tile_pool(name="ysb", bufs=2) as ysb_p,
            tc.tile_pool(name="yt", bufs=1) as yt_p,
            tc.tile_pool(name="gru", bufs=1) as gru_p,
            tc.tile_pool(name="outp", bufs=2) as out_p,
            tc.tile_pool(name="psum", bufs=1, space="PSUM") as ps_p,
        ):
            rg = [list(range(cfg.n_cores))]
            if cfg.xdt == "int4":
                # unpack int4 shard tiles -> fp8 bounce, then AllGather
                r0 = 0
                while r0 < cfg.npc:
                    rw = min(BLOCK, cfg.npc - r0)
                    pk = s_p.tile([BLOCK, T, cph], u8, tag="pk")
                    nc.sync.dma_start(pk[:rw], XSPd[r0:r0 + rw])
                    nib = s_p.tile([BLOCK, T, cph], u8, tag="nib")
                    xup = s_p.tile([BLOCK, T, cp], fp8, tag="xup")
                    nc.vector.tensor_scalar(
                        out=nib[:rw], in0=pk[:rw], scalar1=15, scalar2=None,
                        op0=OP.bitwise_and)
                    nc.vector.tensor_scalar(
                        out=xup[:rw, :, 0:cph], in0=nib[:rw],
                        scalar1=float(INT4_S), scalar2=-8.0 * float(INT4_S),
                        op0=OP.mult, op1=OP.add)
                    nc.vector.tensor_scalar(
                        out=nib[:rw], in0=pk[:rw], scalar1=4, scalar2=None,
                        op0=OP.logical_shift_right)
                    nc.vector.tensor_scalar(
                        out=xup[:rw, :, cph:cp], in0=nib[:rw],
                        scalar1=float(INT4_S), scalar2=-8.0 * float(INT4_S),
                        op0=OP.mult, op1=OP.add)
                    nc.sync.dma_start(xin[r0:r0 + rw], xup[:rw])
                    r0 += rw
                nc.gpsimd.collective_compute(
                    "AllGather", OP.bypass, replica_groups=rg,
                    ins=[xin[:].opt()], outs=[xfull[:].opt()])
            else:
                nc.sync.dma_start(xin0[:], XS0d[:])
                nc.sync.dma_start(xin1[:], XS1d[:])
                nc.gpsimd.collective_compute(
                    "AllGather", OP.bypass, replica_groups=rg,
                    ins=[xin0[:].opt()], outs=[xfull0[:].opt()])
                nc.gpsimd.collective_compute(
                    "AllGather", OP.bypass, replica_groups=rg,
                    ins=[xin1[:].opt()], outs=[xfull1[:].opt()])

            idx_u16 = const_p.tile([BLOCK, nb * S], u16)
            nc.sync.dma_start(idx_u16[:], IDXd[:])
            idx_sb = const_p.tile([BLOCK, nb * S], i32)
            nc.vector.tensor_copy(out=idx_sb[:], in_=idx_u16[:])
            dloc_u8 = const_p.tile([BLOCK, nb * S], mybir.dt.uint8)
            nc.sync.dma_start(dloc_u8[:], DLOCd[:])
            dloc_sb = const_p.tile([BLOCK, nb * S], fp32)
            nc.vector.tensor_copy(out=dloc_sb[:], in_=dloc_u8[:])
            we_bf = const_p.tile([BLOCK, nb * S], bf16)
            nc.sync.dma_start(we_bf[:], WEd[:])
            we_sb = const_p.tile([BLOCK, nb * S], fp32)
            nc.vector.tensor_copy(out=we_sb[:], in_=we_bf[:])
            iota_i = const_p.tile([BLOCK, BLOCK], i32)
            nc.gpsimd.iota(iota_i[:], pattern=[[1, BLOCK]], base=0,
                           channel_multiplier=0)
            iota_bf = const_p.tile([BLOCK, BLOCK], bf16)
            nc.vector.tensor_copy(out=iota_bf[:], in_=iota_i[:])
            wc1_sb = const_p.tile([c1, 3 * O], bf16)
            nc.sync.dma_start(wc1_sb[:], WC1d[:])
            wc2_sb = const_p.tile([c2, 3 * O], bf16)
            nc.sync.dma_start(wc2_sb[:], WC2d[:])
            wl2_sb = const_p.tile([O, 3 * O], bf16)
            nc.sync.dma_start(wl2_sb[:], WL2d[:])
            wo_sb = const_p.tile([O, 1], bf16)
            nc.sync.dma_start(wo_sb[:], WOd[:])
            bias_sb = const_p.tile([O, 4], fp32)
            nc.sync.dma_start(bias_sb[:], BIASd[:])
            ident = const_p.tile([BLOCK, BLOCK], fp32)
            make_identity(nc, ident[:])

            for grp in groups:
                ng = len(grp) * BLOCK          # nodes in group (padded)
                yt1 = yt_p.tile([c1, T, ng], bf16, tag="yt1")
                yt2 = yt_p.tile([max(c2, 1), T, ng], bf16, tag="yt2")

                for bi, blk in enumerate(grp):
                    sdt = fp8 if cfg.xdt == "int4" else xdt
                    s_sb = s_p.tile([BLOCK, S * BLOCK], sdt, tag="smat")
                    for s in range(S):
                        col = blk * S + s
                        nc.vector.tensor_scalar(
                            out=s_sb[:, s * BLOCK:(s + 1) * BLOCK],
                            in0=iota_bf[:],
                            scalar1=dloc_sb[:, col:col + 1],
                            scalar2=we_sb[:, col:col + 1],
                            op0=OP.is_equal, op1=OP.mult)
                    ps_y = ps_p.tile([BLOCK, FF], fp32, tag="psy")
                    gdt = fp8 if cfg.xdt == "int4" else xdt
                    for s in range(S):
                        col = blk * S + s
                        kw = dict(bounds_check=cfg.n_nodes - 1,
                                  oob_is_err=True)
                        g_sb = g_p.tile([BLOCK, FF], gdt, tag="gath")
                        for xf, h0, hw in halves:
                            nc.gpsimd.indirect_dma_start(
                                out=g_sb[:, h0:h0 + hw],
                                out_offset=None,
                                in_=xf[:],
                                in_offset=bass.IndirectOffsetOnAxis(
                                    ap=idx_sb[:, col:col + 1], axis=0),
                                **kw,
                            )
                        for f0, fw in fchunks(FF):
                            nc.tensor.matmul(
                                out=ps_y[:, f0:f0 + fw],
                                lhsT=s_sb[:, s * BLOCK:(s + 1) * BLOCK],
                                rhs=g_sb[:, f0:f0 + fw],
                                start=(s == 0),
                                stop=(s == S - 1),
                            )
                    y_sb = ysb_p.tile([BLOCK, FF], fp32, tag="ysb")
                    nc.vector.tensor_copy(out=y_sb[:], in_=ps_y[:])

                    # per-timestep transposes to channel-major
                    for t in range(T):
                        pt = ps_p.tile([128, MMF], fp32, tag="small")
                        nc.tensor.transpose(
                            out=pt[:c1, :BLOCK],
                            in_=y_sb[:, t * cp:t * cp + c1],
                            identity=ident[:],
                        )
                        if c2 > 0:
                            nc.tensor.transpose(
                                out=pt[:c2, BLOCK:2 * BLOCK],
                                in_=y_sb[:, t * cp + c1:t * cp + cp],
                                identity=ident[:],
                            )
                        nc.scalar.activation(
                            out=yt1[:, t, bi * BLOCK:(bi + 1) * BLOCK],
                            in_=pt[:c1, :BLOCK], func=AF.Copy)
                        if c2 > 0:
                            nc.scalar.activation(
                                out=yt2[:, t, bi * BLOCK:(bi + 1) * BLOCK],
                                in_=pt[:c2, BLOCK:2 * BLOCK], func=AF.Copy)

                # ---- GRU scan over this node group ----
                h_f = gru_p.tile([O, ng], fp32, tag="h")
                h_bf = gru_p.tile([O, ng], bf16, tag="hbf")
                acc = gru_p.tile([O, ng], fp32, tag="acc")
                nc.vector.memset(h_f[:], 0)
                nc.vector.memset(h_bf[:], 0)
                nc.vector.memset(acc[:], 0)

                for t in range(T):
                    def gate_psum(gi, rh_tile=None):
                        gs = slice(gi * O, (gi + 1) * O)
                        pa = ps_p.tile([128, MMF], fp32, tag="small")
                        for f0, fw in fchunks(ng):
                            nc.tensor.matmul(
                                out=pa[:O, f0:f0 + fw],
                                lhsT=wc1_sb[:, gs],
                                rhs=yt1[:, t, f0:f0 + fw],
                                start=True, stop=False)
                            if c2 > 0:
                                nc.tensor.matmul(
                                    out=pa[:O, f0:f0 + fw],
                                    lhsT=wc2_sb[:, gs],
                                    rhs=yt2[:, t, f0:f0 + fw],
                                    start=False, stop=False)
                            hsrc = h_bf if rh_tile is None else rh_tile
                            nc.tensor.matmul(
                                out=pa[:O, f0:f0 + fw],
                                lhsT=wl2_sb[:, gs],
                                rhs=hsrc[:, f0:f0 + fw],
                                start=False, stop=True)
                        return pa

                    pz = gate_psum(0)
                    z_t = gru_p.tile([O, ng], fp32, tag="z")
                    nc.scalar.activation(out=z_t[:], in_=pz[:O, :ng],
                                         func=AF.Sigmoid,
                                         bias=bias_sb[:, 0:1])
                    pr = gate_psum(1)
                    r_t = gru_p.tile([O, ng], fp32, tag="r")
                    nc.scalar.activation(out=r_t[:], in_=pr[:O, :ng],
                                         func=AF.Sigmoid,
                                         bias=bias_sb[:, 1:2])
                    rh = gru_p.tile([O, ng], bf16, tag="rh")
                    nc.vector.tensor_tensor(out=rh[:], in0=r_t[:],
                                            in1=h_f[:], op=OP.mult)
                    ph = gate_psum(2, rh_tile=rh)
                    ht = gru_p.tile([O, ng], fp32, tag="ht")
                    nc.scalar.activation(out=ht[:], in_=ph[:O, :ng], func=AF.Tanh,
                                         bias=bias_sb[:, 2:3])
                    # H = Ht + Z*(H - Ht)
                    d_t = gru_p.tile([O, ng], fp32, tag="d")
                    nc.vector.tensor_tensor(out=d_t[:], in0=h_f[:],
                                            in1=ht[:], op=OP.subtract)
                    nc.vector.tensor_tensor(out=d_t[:], in0=z_t[:],
                                            in1=d_t[:], op=OP.mult)
                    nc.vector.tensor_tensor(out=h_f[:], in0=ht[:],
                                            in1=d_t[:], op=OP.add)
                    # acc += p_t * H
                    p_h = gru_p.tile([O, ng], fp32, tag="phh")
                    nc.scalar.activation(out=p_h[:], in_=h_f[:], func=AF.Copy,
                                         scale=float(probs[t]))
                    nc.vector.tensor_tensor(out=acc[:], in0=acc[:],
                                            in1=p_h[:], op=OP.add)
                    if t < T - 1:
                        nc.scalar.activation(out=h_bf[:], in_=h_f[:],
                                             func=AF.Copy)

                # output head
                acc_bf = gru_p.tile([O, ng], bf16, tag="accbf")
                nc.scalar.activation(out=acc_bf[:], in_=acc[:], func=AF.Copy)
                n0 = grp[0] * BLOCK
                for f0, fw in fchunks(ng):
                    po = ps_p.tile([128, MMF], fp32, tag="small")
                    nc.tensor.matmul(out=po[:1, :fw], lhsT=wo_sb[:],
                                     rhs=acc_bf[:, f0:f0 + fw],
                                     start=True, stop=True)
                    o_sb = out_p.tile([1, MMF], fp32, tag="osb")
                    nc.scalar.activation(out=o_sb[:, :fw], in_=po[:1, :fw],
                                         func=AF.Sigmoid,
                                         bias=bias_sb[0:1, 3:4])
                    w0 = n0 + f0
                    w1 = min(n0 + f0 + fw, cfg.npc)
                    if w1 > w0:
                        nc.sync.dma_start(out=OUTd[:, w0:w1],
                                          in_=o_sb[:, :w1 - w0])

    nc.compile()
    return nc


# ------------------------------- entry points -------------------------------

_CACHE = {}


def _get_nc(cfg, probs):
    k = (cfg.key, tuple(np.round(probs, 8).tolist()))
    if k not in _CACHE:
        _CACHE[k] = build_nc(cfg, probs)
    return _CACHE[k]


_RUNNER_CACHE = {}


def _get_runner(nc, n_cores):
    """Build (once) a reusable jitted SPMD executor for `nc`.

    Mirrors concourse.bass2jax.run_bass_via_pjrt, but caches the jitted
    callable so warm calls skip XLA/BIR re-compilation (which otherwise
    costs seconds per call).
    """
    key = id(nc)
    if key in _RUNNER_CACHE:
        return _RUNNER_CACHE[key]

    import jax
    from jax.sharding import Mesh, PartitionSpec
    from jax.experimental.shard_map import shard_map
    from concourse import bass2jax, mybir

    bass2jax.install_neuronx_cc_hook()
    assert nc.dbg_addr is None or not nc.dbg_callbacks

    partition_name = (nc.partition_id_tensor.name
                      if nc.partition_id_tensor else None)
    in_names, out_names, out_avals = [], [], []
    for alloc in nc.m.functions[0].allocations:
        if not isinstance(alloc, mybir.MemoryLocationSet):
            continue
        name = alloc.memorylocations[0].name
        if alloc.kind == "ExternalInput":
            if name != partition_name and name != (
                    nc.dbg_addr.name if nc.dbg_addr is not None else None):
                in_names.append(name)
        elif alloc.kind == "ExternalOutput":
            out_names.append(name)
            out_avals.append(jax.core.ShapedArray(
                tuple(alloc.tensor_shape), mybir.dt.np(alloc.dtype)))
    n_params = len(in_names)
    body_in_names = list(in_names) + list(out_names)
    if nc.dbg_addr is not None:
        body_in_names.append(nc.dbg_addr.name)
    if partition_name is not None:
        body_in_names.append(partition_name)

    donate = tuple(range(n_params, n_params + len(out_names)))

    def _body(*args):
        operands = list(args)
        if nc.dbg_addr is not None:
            operands.append(jax.numpy.zeros((1, 2), np.uint32))
        if partition_name is not None:
            operands.append(bass2jax.partition_id_tensor())
        outs = bass2jax._bass_exec_p.bind(
            *operands,
            out_avals=tuple(out_avals),
            in_names=tuple(body_in_names),
            out_names=tuple(out_names),
            lowering_input_output_aliases=(),
            sim_require_finite=True,
            sim_require_nnan=True,
            nc=nc,
        )
        return tuple(outs)

    devices = jax.devices()[:n_cores]
    mesh = Mesh(np.asarray(devices), ("core",))
    in_specs = (PartitionSpec("core"),) * (n_params + len(out_names))
    out_specs = (PartitionSpec("core"),) * len(out_names)
    sharded = jax.jit(
        shard_map(_body, mesh=mesh, in_specs=in_specs, out_specs=out_specs,
                  check_rep=False),
        donate_argnums=donate, keep_unused=True)

    concat_cache = {}

    def run(in_maps):
        ck = id(in_maps)
        if ck not in concat_cache:
            concat_cache.clear()
            concat_cache[ck] = [
                np.concatenate([np.asarray(m[name]) for m in in_maps], axis=0)
                for name in in_names
            ]
        concat_in = concat_cache[ck]
        concat_zeros = [
            np.zeros((n_cores * a.shape[0], *a.shape[1:]), a.dtype)
            for a in out_avals
        ]
        out_arrs = sharded(*concat_in, *concat_zeros)
        return [
            {name: np.asarray(out_arrs[i]).reshape(
                n_cores, *out_avals[i].shape)[c]
             for i, name in enumerate(out_names)}
            for c in range(n_cores)
        ]

    _RUNNER_CACHE[key] = run
    return run


def run_device(cfg, per_core, probs, trace=False):
    nc = _get_nc(cfg, probs)
    if trace:
        from concourse.bass_utils import run_bass_kernel_spmd
        res = run_bass_kernel_spmd(nc, per_core,
                                   core_ids=list(range(cfg.n_cores)),
                                   trace=True)
        results = res.results
    else:
        results = _get_runner(nc, cfg.n_cores)(per_core)
        res = None
    outs = [results[c]["OUTd"].reshape(-1)[:cfg.npc]
            for c in range(cfg.n_cores)]
    return np.concatenate(outs), res


def kernel(x, edge_index, y, train_idx, attention,
           W_z, b_z, Wl_z, bl_z, W_r, b_r, Wl_r, bl_r,
           W_h, b_h, Wl_h, bl_h, W_o, b_o):
    x = np.asarray(x)
    y = np.asarray(y, dtype=np.float32)
    train_idx = np.asarray(train_idx)
    cfg, per_core, probs = preprocess(
        x, np.asarray(edge_index), np.asarray(attention),
        W_z, b_z, Wl_z, bl_z, W_r, b_r, Wl_r, bl_r,
        W_h, b_h, Wl_h, bl_h, W_o, b_o)
    full, _ = run_device(cfg, per_core, probs,
                         trace=bool(int(os.environ.get("KTRACE", "0"))))
    y_pred = full[train_idx].astype(np.float32)
    return y_pred, y[train_idx]

